# revision 29
# baseline (speedup 1.0000x reference)
"""BarlowTwins-style loss kernel for Trainium2 (raw Bass), 8-core SPMD.

Math: the reference materializes a (B, D, D) per-sample cross-correlation
tensor, but the loss algebraically reduces to O(B*D) work.  With
z1n/z2n the per-dim (batch-)normalized inputs and per-sample b:
    w    = z1n[b,:] * z2n[b,:]
    R    = sum(w);  P = sum(w^2);  Sa = sum(z1n^2);  Sv = sum(z2n^2)
    a    = z1n[b,b];  v = z2n[b,b];  d = a*v;  g2 = (d-1)^2
    u    = (a*z2n[b,:] - 1)^2;  Q = sum(u^2)
    loss = -2R + (1-l)P + lQ + (1-l)g2^2 + l*d^2 - l*(a^2-Sa)*Sv
           - 3*g2 + (D+1)
  and with -3*g2 = -3(d-1)^2 the final combine is two coefficient dot
  products ([R,P,Q]*c1 and [g2^2, d^2, f=(a^2-Sa)*Sv, 1, d]*c2).

Sharding: data-parallel over batch.  Every core loads the full batch
(as bf16 -- the on-device math is bf16 anyway, so the host pre-rounds
z1||z2 once) to compute per-dim column sums/sumsq locally; an 8-core
all-reduce has a ~10us latency floor, the redundant 512KB load ~2us.

v4 pipeline (31.2us v1 -> 27.3us v3 -> this):
  - z shipped as one bf16 tensor, row-split across the two HWDGE
    queues (4KB packets); cb/cf behind on a GpSimd SWDGE queue.
  - eye one-hots / bias constants built by GpSimd memsets, not DMA.
  - PE warmup: 3 wide + 7 narrow matmuls bridge the DMA window with
    no idle gap, so the real matmuls run at full clock.
  - All 8 column-stat matmuls accumulate into ONE [8,512] PSUM bank
    (one-hot lhsT column per source block); a single [8,512] copy
    (split across DVE+ACT halves) feeds 4 scatter matmuls that place
    sums at rows 0:16 and sumsq at rows 32:48 of one [48,128] PSUM.
  - 1/std = exp(-0.5*ln(var)) on ACT (one table load covers square/
    copy/ln/exp); A-cast on GpSimd, C on DVE, straight out of PSUM.
  - Per-sample: DVE normalize/product chain with P/R/v/Sv/a^2 accums;
    Sa/u/Q squares on ACT in parallel; qfin matmul collapses the
    8 chunk-partitions per sample; finals as two coefficient dots.
"""

import sys
from contextlib import ExitStack

import numpy as np

for _p in ("/opt/trn_rl_repo",):
    if _p not in sys.path:
        sys.path.append(_p)

import concourse.bass as bass
import concourse.mybir as mybir
from concourse.bass_utils import run_bass_kernel_spmd

B, D = 128, 1024
NCORES = 8
SPC = B // NCORES  # 16 samples per core
LAM = 0.005

FP = mybir.dt.float32
BF = mybir.dt.bfloat16
AF = mybir.ActivationFunctionType
AL = mybir.AluOpType

# cb (bf16) column layout
CB_Z1R = 0
CB_Z2R = 128
CB_AMASK = 256
CB_SEL = 384
CB_SEL2 = 512
CB_SCAT = 640   # 4x [8,16] S-scatter + 4x [8,16] Q-scatter lhsT
CB_TOTAL = 768
# cf (fp32) column layout
CF_GSEL = 0
CF_C1 = 16
CF_C2 = 19
CF_TOTAL = 24

K1 = 1.0 / (B * (B - 1.0))
K2 = 1.0 / (B - 1.0)


def build_program():
    nc = bass.Bass("TRN2", debug=False, num_devices=NCORES,
                   detect_race_conditions=False)

    zb_d = nc.dram_tensor("zb_hbm", [128, 2 * D], BF, kind="ExternalInput")
    cb_d = nc.dram_tensor("cb_hbm", [128, CB_TOTAL], BF, kind="ExternalInput")
    cf_d = nc.dram_tensor("cf_hbm", [128, CF_TOTAL], FP, kind="ExternalInput")
    loss_d = nc.dram_tensor("loss", [SPC, 1], FP, kind="ExternalOutput")

    ctx = ExitStack()
    with ctx:
        sem = {n: ctx.enter_context(nc.semaphore(n)) for n in
               ["dzs", "dcb", "dcf", "dout", "spe", "sv", "sa", "sgp"]}

        def sb(name, shape, dtype=FP):
            return ctx.enter_context(nc.sbuf_tensor(name, shape, dtype))

        zb = sb("zb", [128, 2 * D], BF)    # bf16 z1||z2
        sqb = sb("sqb", [128, 2 * D], BF)  # bf16 z^2
        cb = sb("cb", [128, CB_TOTAL], BF)
        cf = sb("cf", [128, CF_TOTAL])
        eyes = sb("eyes", [128, 64], BF)   # 8x [128,8] bf16 one-hot cols
        statq = sb("statq", [8, 512], BF)  # rows 0-3 sums, 4-7 sumsq
        t1 = sb("t1", [16, 128])
        var16 = sb("var16", [16, 128])
        lnv = sb("lnv", [16, 128])
        arec = sb("arec", [16, 128])
        acz = sb("acz", [16, 256], BF)     # A || C bf16
        tn1 = sb("tn1", [128, 128], BF)
        z1n = sb("z1n", [128, 128], BF)
        tn2 = sb("tn2", [128, 128], BF)
        z2n = sb("z2n", [128, 128], BF)
        w = sb("w", [128, 128], BF)
        u = sb("u", [128, 128], BF)
        junk_v = sb("junk_v", [128, 128], BF)
        junk_s = sb("junk_s", [128, 128], BF)
        colsD = sb("colsD", [128, 8])
        negone = sb("negone", [128, 1])
        a_sb = sb("a_sb", [128, 1])
        junk1c = sb("junk1c", [128, 1])
        fin2 = sb("fin2", [16, 8])
        qs = sb("qs", [16, 8])
        g2c = sb("g2c", [16, 1])
        ec = sb("ec", [16, 1])
        acc1c = sb("acc1c", [16, 1])
        acc2c = sb("acc2c", [16, 1])
        loss16 = sb("loss16", [16, 1])
        junkw = sb("junkw", [1, 4])
        junkw2 = sb("junkw2", [1, 4])

        # PSUM
        warmps = ctx.enter_context(nc.psum_tensor("warmps", [128, 512], FP))
        bankAll = ctx.enter_context(nc.psum_tensor("bankAll", [8, 512], FP))
        statSps = ctx.enter_context(nc.psum_tensor("statSps", [16, 128], FP))
        statQps = ctx.enter_context(nc.psum_tensor("statQps", [16, 128], FP))
        psBC1 = ctx.enter_context(nc.psum_tensor("psBC1", [128, 256], FP))
        psBC2 = ctx.enter_context(nc.psum_tensor("psBC2", [128, 256], FP))
        qfinps = ctx.enter_context(nc.psum_tensor("qfinps", [16, 8], FP))

        z1r = cb[:, CB_Z1R:CB_Z1R + 128]
        z2r = cb[:, CB_Z2R:CB_Z2R + 128]
        amask = cb[:, CB_AMASK:CB_AMASK + 128]
        selz1b = cb[0:16, CB_SEL:CB_SEL + 128]
        selz2b = cb[0:16, CB_SEL2:CB_SEL2 + 128]
        gsel = cf[:, CF_GSEL:CF_GSEL + 16]
        c1 = cf[0:16, CF_C1:CF_C1 + 3]
        c2 = cf[0:16, CF_C2:CF_C2 + 5]
        A_zb = acz[:, 0:128]
        C_zb = acz[:, 128:256]
        statS = statSps[:, :]
        statQ = statQps[:, :]

        blk = [slice(i * 512, (i + 1) * 512) for i in range(4)]

        def eye(m):
            return eyes[:, 8 * m:8 * m + 8]

        def scatS(wn):
            return cb[0:8, CB_SCAT + 16 * wn:CB_SCAT + 16 * wn + 16]

        def scatQ(wn):
            return cb[0:8, CB_SCAT + 64 + 16 * wn:CB_SCAT + 64 + 16 * wn + 16]

        with nc.Block() as block:

            @block.sync
            def _(sync):
                sync.dma_start(zb[0:64, :], zb_d[0:64, :]).then_inc(sem["dzs"], 16)
                # output DMA once loss16 is drained
                sync.wait_ge(sem["sv"], 31)
                sync.dma_start(loss_d[:], loss16[:]).then_inc(sem["dout"], 16)

            @block.gpsimd
            def _(gp):
                gp.memset(junkw[:], 2.0).then_inc(sem["sgp"])                   # 1
                gp.memset(eyes[:], 0.0).then_inc(sem["sgp"])                    # 2
                for m in range(8):
                    gp.memset(eyes[:, 9 * m:9 * m + 1], 1.0).then_inc(sem["sgp"])  # 3-10
                gp.memset(negone[:], -1.0).then_inc(sem["sgp"])                 # 11
                gp.memset(fin2[:, 3:4], 1.0).then_inc(sem["sgp"])               # 12
                # SWDGE loads for the constant tables (3rd DMA queue)
                gp.dma_start(cb[:], cb_d[:]).then_inc(sem["dcb"], 16)
                gp.dma_start(cf[:], cf_d[:]).then_inc(sem["dcf"], 16)
                gp.wait_ge(sem["dout"], 16)

            @block.scalar
            def _(act):
                act.dma_start(zb[64:128, :], zb_d[64:128, :]).then_inc(sem["dzs"], 16)
                # preload the ACT function table during the DMA phase
                act.square(junkw2[:], junkw[:]).then_inc(sem["sa"])             # 1
                act.wait_ge(sem["dzs"], 32)
                act.square(sqb[:, blk[0]], zb[:, blk[0]]).then_inc(sem["sa"])   # 2
                act.square(sqb[:, blk[1]], zb[:, blk[1]]).then_inc(sem["sa"])   # 3
                act.copy(junkw2[:], junkw[:]).then_inc(sem["sa"])               # 4 (spacing)
                # t1 = K1 * S^2 straight out of scatter PSUM
                act.wait_ge(sem["spe"], 12)
                act.activation(t1[:], statS, AF.Square,
                               scale=float(np.sqrt(K1))).then_inc(sem["sa"])    # 5
                act.wait_ge(sem["sv"], 4)
                act.activation(lnv[:], var16[:], AF.Ln).then_inc(sem["sa"])     # 6
                act.copy(junkw2[:], junkw[:]).then_inc(sem["sa"])               # 7 (spacing)
                act.activation(arec[:], lnv[:], AF.Exp,
                               scale=-0.5).then_inc(sem["sa"])                  # 8
                # ---- per-sample squares ----
                act.wait_ge(sem["sv"], 8)
                act.activation(junk_s[:], z1n[:], AF.Square,
                               accum_out=colsD[:, 5:6]).then_inc(sem["sa"])     # 9 Sa
                act.wait_ge(sem["sgp"], 11)
                act.wait_ge(sem["sv"], 12)
                act.activation(u[:], z2n[:], AF.Square, bias=negone[:],
                               scale=a_sb[:]).then_inc(sem["sa"])               # 10 u
                act.activation(junk_s[:], u[:], AF.Square,
                               accum_out=colsD[:, 2:3]).then_inc(sem["sa"])     # 11 Q
                act.copy(junkw2[:], junkw[:]).then_inc(sem["sa"])               # 12 (spacing)
                act.wait_ge(sem["spe"], 19)
                act.copy(qs[:], qfinps[:]).then_inc(sem["sa"])                  # 13
                act.wait_ge(sem["sv"], 19)
                act.activation(g2c[:], fin2[:, 4:5], AF.Square,
                               bias=negone[0:16, :]).then_inc(sem["sa"])        # 14

            @block.vector
            def _(dve):
                dve.wait_ge(sem["dzs"], 32)
                dve.scalar_tensor_tensor(
                    sqb[:, blk[2]], zb[:, blk[2]], 1.0, zb[:, blk[2]],
                    op0=AL.bypass, op1=AL.mult).then_inc(sem["sv"])             # 1
                dve.scalar_tensor_tensor(
                    sqb[:, blk[3]], zb[:, blk[3]], 1.0, zb[:, blk[3]],
                    op0=AL.bypass, op1=AL.mult).then_inc(sem["sv"])             # 2
                # stats PSUM -> SBUF (bf16)
                dve.wait_ge(sem["spe"], 8)
                dve.tensor_copy(statq[:], bankAll[:]).then_inc(sem["sv"])       # 3
                # var = K2*Q - t1 (Q straight out of scatter PSUM)
                dve.wait_ge(sem["spe"], 16)
                dve.wait_ge(sem["sa"], 5)
                dve.scalar_tensor_tensor(
                    var16[:], statQ, K2, t1[:],
                    op0=AL.mult, op1=AL.subtract).then_inc(sem["sv"])           # 4
                dve.wait_ge(sem["sa"], 8)
                dve.scalar_tensor_tensor(
                    C_zb, statS, 1.0 / B, arec[:],
                    op0=AL.mult, op1=AL.mult).then_inc(sem["sv"])               # 5
                dve.tensor_copy(A_zb, arec[:]).then_inc(sem["sv"])              # 6
                # ---- normalize + per-sample products ----
                dve.wait_ge(sem["spe"], 17)
                dve.wait_ge(sem["dcb"], 16)
                dve.tensor_tensor(tn1[:], z1r, psBC1[:, 0:128], AL.mult).then_inc(sem["sv"])  # 7
                dve.tensor_tensor(z1n[:], tn1[:], psBC1[:, 128:256],
                                  AL.subtract).then_inc(sem["sv"])              # 7
                dve.scalar_tensor_tensor(
                    junk_v[:], z1n[:], 1.0, amask, op0=AL.bypass, op1=AL.mult,
                    accum_out=colsD[:, 3:4]).then_inc(sem["sv"])                # 8
                dve.wait_ge(sem["spe"], 18)
                dve.tensor_tensor(tn2[:], z2r, psBC2[:, 0:128], AL.mult).then_inc(sem["sv"])  # 9
                dve.tensor_tensor(z2n[:], tn2[:], psBC2[:, 128:256],
                                  AL.subtract).then_inc(sem["sv"])              # 10
                dve.stream_shuffle(a_sb[:], colsD[:, 3:4],
                                   [8 * (i // 8) for i in range(32)]).then_inc(sem["sv"])  # 11
                dve.scalar_tensor_tensor(
                    w[:], z1n[:], 1.0, z2n[:], op0=AL.bypass, op1=AL.mult,
                    accum_out=colsD[:, 0:1]).then_inc(sem["sv"])                # 12 R
                dve.scalar_tensor_tensor(
                    junk_v[:], z2n[:], 1.0, amask, op0=AL.bypass, op1=AL.mult,
                    accum_out=colsD[:, 4:5]).then_inc(sem["sv"])                # 13 v
                dve.scalar_tensor_tensor(
                    junk_v[:], z2n[:], 1.0, z2n[:], op0=AL.bypass, op1=AL.mult,
                    accum_out=colsD[:, 6:7]).then_inc(sem["sv"])                # 14 Sv
                dve.scalar_tensor_tensor(
                    junk_v[:], w[:], 1.0, w[:], op0=AL.bypass, op1=AL.mult,
                    accum_out=colsD[:, 1:2]).then_inc(sem["sv"])                # 15 P
                dve.tensor_tensor(colsD[:, 7:8], colsD[:, 3:4], colsD[:, 3:4],
                                  AL.mult).then_inc(sem["sv"])                  # 16 a^2
                dve.drain().then_inc(sem["sv"])                                 # 17
                # ---- finals ----
                dve.wait_ge(sem["spe"], 19)
                dve.wait_ge(sem["sa"], 13)
                dve.wait_ge(sem["sgp"], 12)
                dve.tensor_tensor(fin2[:, 4:5], qs[:, 3:4], qs[:, 4:5],
                                  AL.mult).then_inc(sem["sv"])                  # 18 d
                dve.tensor_tensor(ec[:], qs[:, 7:8], qs[:, 5:6],
                                  AL.subtract).then_inc(sem["sv"])              # 19 a2-Sa
                dve.scalar_tensor_tensor(
                    fin2[:, 5:8], qs[:, 0:3], 1.0, c1, op0=AL.bypass,
                    op1=AL.mult, accum_out=acc1c[:]).then_inc(sem["sv"])        # 20 acc1
                dve.tensor_tensor(fin2[:, 1:2], fin2[:, 4:5], fin2[:, 4:5],
                                  AL.mult).then_inc(sem["sv"])                  # 21 d^2
                dve.tensor_tensor(fin2[:, 2:3], ec[:], qs[:, 6:7],
                                  AL.mult).then_inc(sem["sv"])                  # 22 f
                dve.wait_ge(sem["sa"], 14)
                dve.tensor_tensor(fin2[:, 0:1], g2c[:], g2c[:],
                                  AL.mult).then_inc(sem["sv"])                  # 23 g2^2
                dve.tensor_tensor(junk1c[:], negone[:], negone[:],
                                  AL.mult).then_inc(sem["sv"])                  # 24 (spacing)
                dve.tensor_tensor(junk1c[:], negone[:], negone[:],
                                  AL.mult).then_inc(sem["sv"])                  # 25 (spacing)
                dve.scalar_tensor_tensor(
                    qs[:, 0:5], fin2[:, 0:5], 1.0, c2, op0=AL.bypass,
                    op1=AL.mult, accum_out=acc2c[:]).then_inc(sem["sv"])        # 26 acc2
                dve.tensor_tensor(junk1c[:], negone[:], negone[:],
                                  AL.mult).then_inc(sem["sv"])                  # 27 (spacing)
                dve.tensor_tensor(junk1c[:], negone[:], negone[:],
                                  AL.mult).then_inc(sem["sv"])                  # 28 (spacing)
                dve.tensor_tensor(loss16[:], acc1c[:], acc2c[:],
                                  AL.add).then_inc(sem["sv"])                   # 29
                dve.drain().then_inc(sem["sv"])                                 # 30

            @block.tensor
            def _(pe):
                # p-state warmups on garbage SBUF -> scratch PSUM: 3 wide to
                # fill the pipe, then narrow trickles to hold the clock high
                # without delaying the first real matmul.
                for i in range(3):
                    pe.matmul(warmps[0:8, :], cb[:, 0:8], cb[:, 128:640],
                              start=True, stop=True,
                              skip_group_check=True)
                for i in range(7):
                    pe.matmul(warmps[0:8, 0:128], cb[:, 0:8], cb[:, 128:256],
                              start=True, stop=True,
                              skip_group_check=True)
                # column sums (rows 0-3), then sumsq (rows 4-7), one bank
                pe.wait_ge(sem["sgp"], 10)
                pe.wait_ge(sem["dzs"], 32)
                for m in range(4):
                    pe.matmul(bankAll[:], eye(m), zb[:, blk[m]],
                              start=(m == 0), stop=False,
                              skip_group_check=True).then_inc(sem["spe"])       # 1-4
                pe.wait_ge(sem["sa"], 2)
                pe.matmul(bankAll[:], eye(4), sqb[:, blk[0]], start=False, stop=False,
                          skip_group_check=True).then_inc(sem["spe"])           # 5
                pe.wait_ge(sem["sa"], 3)
                pe.matmul(bankAll[:], eye(5), sqb[:, blk[1]], start=False, stop=False,
                          skip_group_check=True).then_inc(sem["spe"])           # 6
                pe.wait_ge(sem["sv"], 1)
                pe.matmul(bankAll[:], eye(6), sqb[:, blk[2]], start=False, stop=False,
                          skip_group_check=True).then_inc(sem["spe"])           # 7
                pe.wait_ge(sem["sv"], 2)
                pe.matmul(bankAll[:], eye(7), sqb[:, blk[3]], start=False, stop=True,
                          skip_group_check=True).then_inc(sem["spe"])           # 8
                # scatter [8,512] -> 2x [16,128]
                pe.wait_ge(sem["dcb"], 16)
                pe.wait_ge(sem["sv"], 3)
                pe.wait_ge(sem["sa"], 4)
                for wn in range(4):
                    pe.matmul(statSps[:], scatS(wn),
                              statq[:, 128 * wn:128 * wn + 128],
                              start=(wn == 0), stop=(wn == 3),
                              skip_group_check=True).then_inc(sem["spe"])       # 9-12
                for wn in range(4):
                    pe.matmul(statQps[:], scatQ(wn),
                              statq[:, 128 * wn:128 * wn + 128],
                              start=(wn == 0), stop=(wn == 3),
                              skip_group_check=True).then_inc(sem["spe"])       # 13-16
                # broadcast A||C to the (sample, chunk) layout
                pe.wait_ge(sem["sv"], 6)
                pe.matmul(psBC1[:], selz1b, acz[:], start=True,
                          stop=True).then_inc(sem["spe"])                       # 17
                pe.matmul(psBC2[:], selz2b, acz[:], start=True, stop=True,
                          skip_group_check=True).then_inc(sem["spe"])           # 18
                # group-reduce: collapse 8 chunk-rows per sample
                pe.wait_ge(sem["dcf"], 16)
                pe.wait_ge(sem["sv"], 18)
                pe.wait_ge(sem["sa"], 12)
                pe.matmul(qfinps[:], gsel, colsD[:], start=True,
                          stop=True).then_inc(sem["spe"])                       # 19

    return nc


def _host_inputs(z1, z2):
    """Per-core input maps (sharding glue)."""
    import ml_dtypes

    z1 = np.ascontiguousarray(z1, np.float32)
    z2 = np.ascontiguousarray(z2, np.float32)
    zb_full = np.concatenate([z1, z2], axis=1).astype(ml_dtypes.bfloat16)

    cb_base = np.zeros((128, CB_TOTAL), np.float32)
    for m in range(128):
        cb_base[m % 8, CB_SEL + m] = 1.0        # selz1b (reads A/C rows 0-7)
        cb_base[8 + m % 8, CB_SEL2 + m] = 1.0   # selz2b (reads A/C rows 8-15)
    for wn in range(4):
        for g in range(4):
            # scatter lhsT_w: S block g / Q block g -> chunk row 4g+w
            cb_base[g, CB_SCAT + 16 * wn + 4 * g + wn] = 1.0
            cb_base[4 + g, CB_SCAT + 64 + 16 * wn + 4 * g + wn] = 1.0

    cf_base = np.zeros((128, CF_TOTAL), np.float32)
    for m in range(128):
        cf_base[m, CF_GSEL + m // 8] = 1.0      # gsel
    cf_base[0:16, CF_C1:CF_C1 + 3] = np.array(
        [-2.0, 1.0 - LAM, LAM], np.float32)
    cf_base[0:16, CF_C2:CF_C2 + 5] = np.array(
        [1.0 - LAM, LAM - 3.0, -LAM, float(D - 2), 6.0], np.float32)

    in_maps = []
    for c in range(NCORES):
        rows = slice(c * SPC, (c + 1) * SPC)
        cbc = cb_base.copy()
        cbc[:, CB_Z1R:CB_Z1R + 128] = z1[rows].reshape(128, 128)
        cbc[:, CB_Z2R:CB_Z2R + 128] = z2[rows].reshape(128, 128)
        for s in range(SPC):
            cbc[s * 8, CB_AMASK + c * SPC + s] = 1.0
        in_maps.append({
            "zb_hbm": zb_full,
            "cb_hbm": np.ascontiguousarray(cbc.astype(ml_dtypes.bfloat16)),
            "cf_hbm": np.ascontiguousarray(cf_base),
        })
    return in_maps


_cached_nc = None


def run(z1, z2, trace=False, **kwargs):
    global _cached_nc
    if _cached_nc is None:
        _cached_nc = build_program()
    in_maps = _host_inputs(z1, z2)
    res = run_bass_kernel_spmd(
        _cached_nc, in_maps, core_ids=list(range(NCORES)), trace=trace, **kwargs)
    out = np.concatenate([res.results[c]["loss"][:, 0] for c in range(NCORES)])
    return out.astype(np.float32), res


def kernel(z1, z2):
    out, _ = run(z1, z2, trace=False)
    return out


# revision 32
# speedup vs baseline: 1.0479x; 1.0479x over previous
"""BarlowTwins-style loss kernel for Trainium2 (raw Bass), 8-core SPMD.

Math: the reference materializes a (B, D, D) per-sample cross-correlation
tensor, but the loss algebraically reduces to O(B*D) work.  With
z1n/z2n the per-dim (batch-)normalized inputs and per-sample b:
    w    = z1n[b,:] * z2n[b,:]
    R    = sum(w);  P = sum(w^2);  Sa = sum(z1n^2);  Sv = sum(z2n^2)
    a    = z1n[b,b];  v = z2n[b,b];  d = a*v;  g2 = (d-1)^2
    u    = (a*z2n[b,:] - 1)^2;  Q = sum(u^2)
    loss = -2R + (1-l)P + lQ + (1-l)g2^2 + l*d^2 - l*(a^2-Sa)*Sv
           - 3*g2 + (D+1)
  and with -3*g2 = -3(d-1)^2 the final combine is two coefficient dot
  products ([R,P,Q]*c1 and [g2^2, d^2, f=(a^2-Sa)*Sv, 1, d]*c2).

Sharding: data-parallel over batch.  Every core loads the full batch
(as bf16 -- the on-device math is bf16 anyway, so the host pre-rounds
z1||z2 once) to compute per-dim column sums/sumsq locally; an 8-core
all-reduce has a ~10us latency floor, the redundant 512KB load ~2us.

v4 pipeline (31.2us v1 -> 27.3us v3 -> this):
  - z shipped as one bf16 tensor, row-split across the two HWDGE
    queues (4KB packets); cb/cf behind on a GpSimd SWDGE queue.
  - eye one-hots / bias constants built by GpSimd memsets, not DMA.
  - PE warmup: 3 wide + 7 narrow matmuls bridge the DMA window with
    no idle gap, so the real matmuls run at full clock.
  - All 8 column-stat matmuls accumulate into ONE [8,512] PSUM bank
    (one-hot lhsT column per source block); a single [8,512] copy
    (split across DVE+ACT halves) feeds 4 scatter matmuls that place
    sums at rows 0:16 and sumsq at rows 32:48 of one [48,128] PSUM.
  - 1/std = exp(-0.5*ln(var)) on ACT (one table load covers square/
    copy/ln/exp); A-cast on GpSimd, C on DVE, straight out of PSUM.
  - Per-sample: DVE normalize/product chain with P/R/v/Sv/a^2 accums;
    Sa/u/Q squares on ACT in parallel; qfin matmul collapses the
    8 chunk-partitions per sample; finals as two coefficient dots.
"""

import sys
from contextlib import ExitStack

import numpy as np

for _p in ("/opt/trn_rl_repo",):
    if _p not in sys.path:
        sys.path.append(_p)

import concourse.bass as bass
import concourse.mybir as mybir
from concourse.bass_utils import run_bass_kernel_spmd

B, D = 128, 1024
NCORES = 8
SPC = B // NCORES  # 16 samples per core
LAM = 0.005

FP = mybir.dt.float32
BF = mybir.dt.bfloat16
AF = mybir.ActivationFunctionType
AL = mybir.AluOpType

# cb (bf16) column layout
CB_Z1R = 0
CB_Z2R = 128
CB_AMASK = 256
CB_SEL = 384
CB_SEL2 = 512
CB_SCAT = 640   # 4x [8,16] S-scatter + 4x [8,16] Q-scatter lhsT
CB_TOTAL = 768
# cf (fp32) column layout
CF_GSEL = 0
CF_C1 = 16
CF_C2 = 19
CF_TOTAL = 24

K1 = 1.0 / (B * (B - 1.0))
K2 = 1.0 / (B - 1.0)


def build_program():
    nc = bass.Bass("TRN2", debug=False, num_devices=NCORES,
                   detect_race_conditions=False)

    zb_d = nc.dram_tensor("zb_hbm", [128, 2 * D], BF, kind="ExternalInput")
    cb_d = nc.dram_tensor("cb_hbm", [128, CB_TOTAL], BF, kind="ExternalInput")
    cf_d = nc.dram_tensor("cf_hbm", [128, CF_TOTAL], FP, kind="ExternalInput")
    loss_d = nc.dram_tensor("loss", [SPC, 1], FP, kind="ExternalOutput")

    ctx = ExitStack()
    with ctx:
        sem = {n: ctx.enter_context(nc.semaphore(n)) for n in
               ["dzs", "dza", "dcb", "dcf", "dout", "spe", "sv", "sa", "sgp"]}

        def sb(name, shape, dtype=FP):
            return ctx.enter_context(nc.sbuf_tensor(name, shape, dtype))

        zb = sb("zb", [128, 2 * D], BF)    # bf16 z1||z2
        sqb = sb("sqb", [128, 2 * D], BF)  # bf16 z^2
        cb = sb("cb", [128, CB_TOTAL], BF)
        cf = sb("cf", [128, CF_TOTAL])
        eyes = sb("eyes", [128, 64], BF)   # 8x [128,8] bf16 one-hot cols
        statq = sb("statq", [8, 512], BF)  # rows 0-3 sums, 4-7 sumsq
        t1 = sb("t1", [16, 128])
        var16 = sb("var16", [16, 128])
        lnv = sb("lnv", [16, 128])
        arec = sb("arec", [16, 128])
        acz = sb("acz", [16, 256], BF)     # A || C bf16
        tn1 = sb("tn1", [128, 128], BF)
        z1n = sb("z1n", [128, 128], BF)
        tn2 = sb("tn2", [128, 128], BF)
        z2n = sb("z2n", [128, 128], BF)
        w = sb("w", [128, 128], BF)
        u = sb("u", [128, 128], BF)
        junk_v = sb("junk_v", [128, 128], BF)
        junk_s = sb("junk_s", [128, 128], BF)
        colsD = sb("colsD", [128, 8])
        negone = sb("negone", [128, 1])
        a_sb = sb("a_sb", [128, 1])
        junk1c = sb("junk1c", [128, 1])
        fin2 = sb("fin2", [16, 8])
        qs = sb("qs", [16, 8])
        g2c = sb("g2c", [16, 1])
        ec = sb("ec", [16, 1])
        acc1c = sb("acc1c", [16, 1])
        acc2c = sb("acc2c", [16, 1])
        loss16 = sb("loss16", [16, 1])
        junkw = sb("junkw", [1, 4])
        junkw2 = sb("junkw2", [1, 4])

        # PSUM
        warmps = ctx.enter_context(nc.psum_tensor("warmps", [128, 512], FP))
        bankAll = ctx.enter_context(nc.psum_tensor("bankAll", [8, 512], FP))
        statSps = ctx.enter_context(nc.psum_tensor("statSps", [16, 128], FP))
        statQps = ctx.enter_context(nc.psum_tensor("statQps", [16, 128], FP))
        psBC1 = ctx.enter_context(nc.psum_tensor("psBC1", [128, 256], FP))
        psBC2 = ctx.enter_context(nc.psum_tensor("psBC2", [128, 256], FP))
        qfinps = ctx.enter_context(nc.psum_tensor("qfinps", [16, 8], FP))

        z1r = cb[:, CB_Z1R:CB_Z1R + 128]
        z2r = cb[:, CB_Z2R:CB_Z2R + 128]
        amask = cb[:, CB_AMASK:CB_AMASK + 128]
        selz1b = cb[0:16, CB_SEL:CB_SEL + 128]
        selz2b = cb[0:16, CB_SEL2:CB_SEL2 + 128]
        gsel = cf[:, CF_GSEL:CF_GSEL + 16]
        c1 = cf[0:16, CF_C1:CF_C1 + 3]
        c2 = cf[0:16, CF_C2:CF_C2 + 5]
        A_zb = acz[:, 0:128]
        C_zb = acz[:, 128:256]
        statS = statSps[:, :]
        statQ = statQps[:, :]

        blk = [slice(i * 512, (i + 1) * 512) for i in range(4)]

        def eye(m):
            return eyes[:, 8 * m:8 * m + 8]

        def scatS(wn):
            return cb[0:8, CB_SCAT + 16 * wn:CB_SCAT + 16 * wn + 16]

        def scatQ(wn):
            return cb[0:8, CB_SCAT + 64 + 16 * wn:CB_SCAT + 64 + 16 * wn + 16]

        with nc.Block() as block:

            @block.sync
            def _(sync):
                sync.dma_start(zb[:, 0:1024], zb_d[:, 0:1024]).then_inc(sem["dzs"], 16)
                # output DMA once loss16 is drained
                sync.wait_ge(sem["sv"], 31)
                sync.dma_start(loss_d[:], loss16[:]).then_inc(sem["dout"], 16)

            @block.gpsimd
            def _(gp):
                gp.memset(junkw[:], 2.0).then_inc(sem["sgp"])                   # 1
                gp.memset(eyes[:], 0.0).then_inc(sem["sgp"])                    # 2
                for m in range(8):
                    gp.memset(eyes[:, 9 * m:9 * m + 1], 1.0).then_inc(sem["sgp"])  # 3-10
                gp.memset(negone[:], -1.0).then_inc(sem["sgp"])                 # 11
                gp.memset(fin2[:, 3:4], 1.0).then_inc(sem["sgp"])               # 12
                # SWDGE loads for the constant tables (3rd DMA queue)
                gp.dma_start(cb[:], cb_d[:]).then_inc(sem["dcb"], 16)
                gp.dma_start(cf[:], cf_d[:]).then_inc(sem["dcf"], 16)
                gp.wait_ge(sem["dout"], 16)

            @block.scalar
            def _(act):
                act.dma_start(zb[:, 1024:2048], zb_d[:, 1024:2048]).then_inc(sem["dza"], 16)
                # preload the ACT function table during the DMA phase
                act.square(junkw2[:], junkw[:]).then_inc(sem["sa"])             # 1
                act.wait_ge(sem["dzs"], 16)
                act.square(sqb[:, blk[0]], zb[:, blk[0]]).then_inc(sem["sa"])   # 2
                act.square(sqb[:, blk[1]], zb[:, blk[1]]).then_inc(sem["sa"])   # 3
                act.copy(junkw2[:], junkw[:]).then_inc(sem["sa"])               # 4 (spacing)
                # t1 = K1 * S^2 straight out of scatter PSUM
                act.wait_ge(sem["spe"], 12)
                act.activation(t1[:], statS, AF.Square,
                               scale=float(np.sqrt(K1))).then_inc(sem["sa"])    # 5
                act.wait_ge(sem["sv"], 4)
                act.activation(lnv[:], var16[:], AF.Ln).then_inc(sem["sa"])     # 5
                act.copy(junkw2[:], junkw[:]).then_inc(sem["sa"])               # 6 (spacing)
                act.activation(arec[:], lnv[:], AF.Exp,
                               scale=-0.5).then_inc(sem["sa"])                  # 7
                # ---- per-sample squares ----
                act.wait_ge(sem["sv"], 8)
                act.activation(junk_s[:], z1n[:], AF.Square,
                               accum_out=colsD[:, 5:6]).then_inc(sem["sa"])     # 8 Sa
                act.wait_ge(sem["sgp"], 11)
                act.wait_ge(sem["sv"], 12)
                act.activation(u[:], z2n[:], AF.Square, bias=negone[:],
                               scale=a_sb[:]).then_inc(sem["sa"])               # 9 u
                act.activation(junk_s[:], u[:], AF.Square,
                               accum_out=colsD[:, 2:3]).then_inc(sem["sa"])     # 10 Q
                act.copy(junkw2[:], junkw[:]).then_inc(sem["sa"])               # 11 (spacing)
                act.wait_ge(sem["spe"], 19)
                act.copy(qs[:], qfinps[:]).then_inc(sem["sa"])                  # 12
                act.wait_ge(sem["sv"], 19)
                act.activation(g2c[:], fin2[:, 4:5], AF.Square,
                               bias=negone[0:16, :]).then_inc(sem["sa"])        # 13

            @block.vector
            def _(dve):
                dve.wait_ge(sem["dza"], 16)
                dve.scalar_tensor_tensor(
                    sqb[:, blk[2]], zb[:, blk[2]], 1.0, zb[:, blk[2]],
                    op0=AL.bypass, op1=AL.mult).then_inc(sem["sv"])             # 1
                dve.scalar_tensor_tensor(
                    sqb[:, blk[3]], zb[:, blk[3]], 1.0, zb[:, blk[3]],
                    op0=AL.bypass, op1=AL.mult).then_inc(sem["sv"])             # 2
                # stats PSUM -> SBUF (bf16)
                dve.wait_ge(sem["spe"], 8)
                dve.tensor_copy(statq[:], bankAll[:]).then_inc(sem["sv"])       # 3
                # var = K2*Q - t1 (Q straight out of scatter PSUM)
                dve.wait_ge(sem["spe"], 16)
                dve.wait_ge(sem["sa"], 5)
                dve.scalar_tensor_tensor(
                    var16[:], statQ, K2, t1[:],
                    op0=AL.mult, op1=AL.subtract).then_inc(sem["sv"])           # 4
                dve.wait_ge(sem["sa"], 8)
                dve.scalar_tensor_tensor(
                    C_zb, statS, 1.0 / B, arec[:],
                    op0=AL.mult, op1=AL.mult).then_inc(sem["sv"])               # 5
                dve.tensor_copy(A_zb, arec[:]).then_inc(sem["sv"])              # 6
                # ---- normalize + per-sample products ----
                dve.wait_ge(sem["spe"], 17)
                dve.wait_ge(sem["dcb"], 16)
                dve.tensor_tensor(tn1[:], z1r, psBC1[:, 0:128], AL.mult).then_inc(sem["sv"])  # 7
                dve.tensor_tensor(z1n[:], tn1[:], psBC1[:, 128:256],
                                  AL.subtract).then_inc(sem["sv"])              # 7
                dve.scalar_tensor_tensor(
                    junk_v[:], z1n[:], 1.0, amask, op0=AL.bypass, op1=AL.mult,
                    accum_out=colsD[:, 3:4]).then_inc(sem["sv"])                # 8
                dve.wait_ge(sem["spe"], 18)
                dve.tensor_tensor(tn2[:], z2r, psBC2[:, 0:128], AL.mult).then_inc(sem["sv"])  # 9
                dve.tensor_tensor(z2n[:], tn2[:], psBC2[:, 128:256],
                                  AL.subtract).then_inc(sem["sv"])              # 10
                dve.stream_shuffle(a_sb[:], colsD[:, 3:4],
                                   [8 * (i // 8) for i in range(32)]).then_inc(sem["sv"])  # 11
                dve.scalar_tensor_tensor(
                    w[:], z1n[:], 1.0, z2n[:], op0=AL.bypass, op1=AL.mult,
                    accum_out=colsD[:, 0:1]).then_inc(sem["sv"])                # 12 R
                dve.scalar_tensor_tensor(
                    junk_v[:], z2n[:], 1.0, amask, op0=AL.bypass, op1=AL.mult,
                    accum_out=colsD[:, 4:5]).then_inc(sem["sv"])                # 13 v
                dve.scalar_tensor_tensor(
                    junk_v[:], z2n[:], 1.0, z2n[:], op0=AL.bypass, op1=AL.mult,
                    accum_out=colsD[:, 6:7]).then_inc(sem["sv"])                # 14 Sv
                dve.scalar_tensor_tensor(
                    junk_v[:], w[:], 1.0, w[:], op0=AL.bypass, op1=AL.mult,
                    accum_out=colsD[:, 1:2]).then_inc(sem["sv"])                # 15 P
                dve.tensor_tensor(colsD[:, 7:8], colsD[:, 3:4], colsD[:, 3:4],
                                  AL.mult).then_inc(sem["sv"])                  # 16 a^2
                dve.drain().then_inc(sem["sv"])                                 # 17
                # ---- finals ----
                dve.wait_ge(sem["spe"], 19)
                dve.wait_ge(sem["sa"], 13)
                dve.wait_ge(sem["sgp"], 12)
                dve.tensor_tensor(fin2[:, 4:5], qs[:, 3:4], qs[:, 4:5],
                                  AL.mult).then_inc(sem["sv"])                  # 18 d
                dve.tensor_tensor(ec[:], qs[:, 7:8], qs[:, 5:6],
                                  AL.subtract).then_inc(sem["sv"])              # 19 a2-Sa
                dve.scalar_tensor_tensor(
                    fin2[:, 5:8], qs[:, 0:3], 1.0, c1, op0=AL.bypass,
                    op1=AL.mult, accum_out=acc1c[:]).then_inc(sem["sv"])        # 20 acc1
                dve.tensor_tensor(fin2[:, 1:2], fin2[:, 4:5], fin2[:, 4:5],
                                  AL.mult).then_inc(sem["sv"])                  # 21 d^2
                dve.tensor_tensor(fin2[:, 2:3], ec[:], qs[:, 6:7],
                                  AL.mult).then_inc(sem["sv"])                  # 22 f
                dve.wait_ge(sem["sa"], 14)
                dve.tensor_tensor(fin2[:, 0:1], g2c[:], g2c[:],
                                  AL.mult).then_inc(sem["sv"])                  # 23 g2^2
                dve.tensor_tensor(junk1c[:], negone[:], negone[:],
                                  AL.mult).then_inc(sem["sv"])                  # 24 (spacing)
                dve.tensor_tensor(junk1c[:], negone[:], negone[:],
                                  AL.mult).then_inc(sem["sv"])                  # 25 (spacing)
                dve.scalar_tensor_tensor(
                    qs[:, 0:5], fin2[:, 0:5], 1.0, c2, op0=AL.bypass,
                    op1=AL.mult, accum_out=acc2c[:]).then_inc(sem["sv"])        # 26 acc2
                dve.tensor_tensor(junk1c[:], negone[:], negone[:],
                                  AL.mult).then_inc(sem["sv"])                  # 27 (spacing)
                dve.tensor_tensor(junk1c[:], negone[:], negone[:],
                                  AL.mult).then_inc(sem["sv"])                  # 28 (spacing)
                dve.tensor_tensor(loss16[:], acc1c[:], acc2c[:],
                                  AL.add).then_inc(sem["sv"])                   # 29
                dve.drain().then_inc(sem["sv"])                                 # 30

            @block.tensor
            def _(pe):
                # p-state warmups on garbage SBUF -> scratch PSUM: 3 wide to
                # fill the pipe, then narrow trickles to hold the clock high
                # without delaying the first real matmul.
                for i in range(3):
                    pe.matmul(warmps[0:8, :], cb[:, 0:8], cb[:, 128:640],
                              start=True, stop=True,
                              skip_group_check=True)
                for i in range(7):
                    pe.matmul(warmps[0:8, 0:128], cb[:, 0:8], cb[:, 128:256],
                              start=True, stop=True,
                              skip_group_check=True)
                # column sums (rows 0-3) + sumsq (rows 4-7), one bank
                pe.wait_ge(sem["sgp"], 10)
                pe.wait_ge(sem["dzs"], 16)
                pe.matmul(bankAll[:], eye(0), zb[:, blk[0]], start=True, stop=False,
                          skip_group_check=True).then_inc(sem["spe"])           # 1
                pe.matmul(bankAll[:], eye(1), zb[:, blk[1]], start=False, stop=False,
                          skip_group_check=True).then_inc(sem["spe"])           # 2
                pe.wait_ge(sem["dza"], 16)
                pe.matmul(bankAll[:], eye(2), zb[:, blk[2]], start=False, stop=False,
                          skip_group_check=True).then_inc(sem["spe"])           # 3
                pe.matmul(bankAll[:], eye(3), zb[:, blk[3]], start=False, stop=False,
                          skip_group_check=True).then_inc(sem["spe"])           # 4
                pe.wait_ge(sem["sa"], 2)
                pe.matmul(bankAll[:], eye(4), sqb[:, blk[0]], start=False, stop=False,
                          skip_group_check=True).then_inc(sem["spe"])           # 5
                pe.wait_ge(sem["sa"], 3)
                pe.matmul(bankAll[:], eye(5), sqb[:, blk[1]], start=False, stop=False,
                          skip_group_check=True).then_inc(sem["spe"])           # 6
                pe.wait_ge(sem["sv"], 1)
                pe.matmul(bankAll[:], eye(6), sqb[:, blk[2]], start=False, stop=False,
                          skip_group_check=True).then_inc(sem["spe"])           # 7
                pe.wait_ge(sem["sv"], 2)
                pe.matmul(bankAll[:], eye(7), sqb[:, blk[3]], start=False, stop=True,
                          skip_group_check=True).then_inc(sem["spe"])           # 8
                # scatter [8,512] -> 2x [16,128]
                pe.wait_ge(sem["dcb"], 16)
                pe.wait_ge(sem["sv"], 3)
                pe.wait_ge(sem["sa"], 4)
                for wn in range(4):
                    pe.matmul(statSps[:], scatS(wn),
                              statq[:, 128 * wn:128 * wn + 128],
                              start=(wn == 0), stop=(wn == 3),
                              skip_group_check=True).then_inc(sem["spe"])       # 9-12
                for wn in range(4):
                    pe.matmul(statQps[:], scatQ(wn),
                              statq[:, 128 * wn:128 * wn + 128],
                              start=(wn == 0), stop=(wn == 3),
                              skip_group_check=True).then_inc(sem["spe"])       # 13-16
                # broadcast A||C to the (sample, chunk) layout
                pe.wait_ge(sem["sv"], 6)
                pe.matmul(psBC1[:], selz1b, acz[:], start=True,
                          stop=True).then_inc(sem["spe"])                       # 17
                pe.matmul(psBC2[:], selz2b, acz[:], start=True, stop=True,
                          skip_group_check=True).then_inc(sem["spe"])           # 18
                # group-reduce: collapse 8 chunk-rows per sample
                pe.wait_ge(sem["dcf"], 16)
                pe.wait_ge(sem["sv"], 18)
                pe.wait_ge(sem["sa"], 12)
                pe.matmul(qfinps[:], gsel, colsD[:], start=True,
                          stop=True).then_inc(sem["spe"])                       # 19

    return nc


def _host_inputs(z1, z2):
    """Per-core input maps (sharding glue)."""
    import ml_dtypes

    z1 = np.ascontiguousarray(z1, np.float32)
    z2 = np.ascontiguousarray(z2, np.float32)
    zb_full = np.concatenate([z1, z2], axis=1).astype(ml_dtypes.bfloat16)

    cb_base = np.zeros((128, CB_TOTAL), np.float32)
    for m in range(128):
        cb_base[m % 8, CB_SEL + m] = 1.0        # selz1b (reads A/C rows 0-7)
        cb_base[8 + m % 8, CB_SEL2 + m] = 1.0   # selz2b (reads A/C rows 8-15)
    for wn in range(4):
        for g in range(4):
            # scatter lhsT_w: S block g / Q block g -> chunk row 4g+w
            cb_base[g, CB_SCAT + 16 * wn + 4 * g + wn] = 1.0
            cb_base[4 + g, CB_SCAT + 64 + 16 * wn + 4 * g + wn] = 1.0

    cf_base = np.zeros((128, CF_TOTAL), np.float32)
    for m in range(128):
        cf_base[m, CF_GSEL + m // 8] = 1.0      # gsel
    cf_base[0:16, CF_C1:CF_C1 + 3] = np.array(
        [-2.0, 1.0 - LAM, LAM], np.float32)
    cf_base[0:16, CF_C2:CF_C2 + 5] = np.array(
        [1.0 - LAM, LAM - 3.0, -LAM, float(D - 2), 6.0], np.float32)

    in_maps = []
    for c in range(NCORES):
        rows = slice(c * SPC, (c + 1) * SPC)
        cbc = cb_base.copy()
        cbc[:, CB_Z1R:CB_Z1R + 128] = z1[rows].reshape(128, 128)
        cbc[:, CB_Z2R:CB_Z2R + 128] = z2[rows].reshape(128, 128)
        for s in range(SPC):
            cbc[s * 8, CB_AMASK + c * SPC + s] = 1.0
        in_maps.append({
            "zb_hbm": zb_full,
            "cb_hbm": np.ascontiguousarray(cbc.astype(ml_dtypes.bfloat16)),
            "cf_hbm": np.ascontiguousarray(cf_base),
        })
    return in_maps


_cached_nc = None


def run(z1, z2, trace=False, **kwargs):
    global _cached_nc
    if _cached_nc is None:
        _cached_nc = build_program()
    in_maps = _host_inputs(z1, z2)
    res = run_bass_kernel_spmd(
        _cached_nc, in_maps, core_ids=list(range(NCORES)), trace=trace, **kwargs)
    out = np.concatenate([res.results[c]["loss"][:, 0] for c in range(NCORES)])
    return out.astype(np.float32), res


def kernel(z1, z2):
    out, _ = run(z1, z2, trace=False)
    return out


# revision 33
# speedup vs baseline: 1.1244x; 1.0730x over previous
"""BarlowTwins-style loss kernel for Trainium2 (raw Bass), 8-core SPMD.

Math: the reference materializes a (B, D, D) per-sample cross-correlation
tensor, but the loss algebraically reduces to O(B*D) work.  With
z1n/z2n the per-dim (batch-)normalized inputs and per-sample b:
    w    = z1n[b,:] * z2n[b,:]
    R    = sum(w);  P = sum(w^2);  Sa = sum(z1n^2);  Sv = sum(z2n^2)
    a    = z1n[b,b];  v = z2n[b,b];  d = a*v;  g2 = (d-1)^2
    u    = (a*z2n[b,:] - 1)^2;  Q = sum(u^2)
    loss = -2R + (1-l)P + lQ + (1-l)g2^2 + l*d^2 - l*(a^2-Sa)*Sv
           - 3*g2 + (D+1)
  and with -3*g2 = -3(d-1)^2 the final combine is two coefficient dot
  products ([R,P,Q]*c1 and [g2^2, d^2, f=(a^2-Sa)*Sv, 1, d]*c2).

Sharding: data-parallel over batch.  Every core loads the full batch
(as bf16 -- the on-device math is bf16 anyway, so the host pre-rounds
z1||z2 once) to compute per-dim column sums/sumsq locally; an 8-core
all-reduce has a ~10us latency floor, the redundant 512KB load ~2us.

v4 pipeline (31.2us v1 -> 27.3us v3 -> this):
  - z shipped as one bf16 tensor, row-split across the two HWDGE
    queues (4KB packets); cb/cf behind on a GpSimd SWDGE queue.
  - eye one-hots / bias constants built by GpSimd memsets, not DMA.
  - PE warmup: 3 wide + 7 narrow matmuls bridge the DMA window with
    no idle gap, so the real matmuls run at full clock.
  - All 8 column-stat matmuls accumulate into ONE [8,512] PSUM bank
    (one-hot lhsT column per source block); a single [8,512] copy
    (split across DVE+ACT halves) feeds 4 scatter matmuls that place
    sums at rows 0:16 and sumsq at rows 32:48 of one [48,128] PSUM.
  - 1/std = exp(-0.5*ln(var)) on ACT (one table load covers square/
    copy/ln/exp); A-cast on GpSimd, C on DVE, straight out of PSUM.
  - Per-sample: DVE normalize/product chain with P/R/v/Sv/a^2 accums;
    Sa/u/Q squares on ACT in parallel; qfin matmul collapses the
    8 chunk-partitions per sample; finals as two coefficient dots.
"""

import sys
from contextlib import ExitStack

import numpy as np

for _p in ("/opt/trn_rl_repo",):
    if _p not in sys.path:
        sys.path.append(_p)

import concourse.bass as bass
import concourse.mybir as mybir
from concourse.bass_utils import run_bass_kernel_spmd

B, D = 128, 1024
NCORES = 8
SPC = B // NCORES  # 16 samples per core
LAM = 0.005

FP = mybir.dt.float32
BF = mybir.dt.bfloat16
AF = mybir.ActivationFunctionType
AL = mybir.AluOpType

# cb (bf16) column layout
CB_Z1R = 0
CB_Z2R = 128
CB_AMASK = 256
CB_SEL = 384
CB_SEL2 = 512
CB_SCAT = 640   # 4x [8,16] S-scatter + 4x [8,16] Q-scatter lhsT
CB_TOTAL = 768
# cf (fp32) column layout
CF_GSEL = 0
CF_C1 = 16
CF_C2 = 19
CF_TOTAL = 24

K1 = 1.0 / (B * (B - 1.0))
K2 = 1.0 / (B - 1.0)


def build_program():
    nc = bass.Bass("TRN2", debug=False, num_devices=NCORES,
                   detect_race_conditions=False)

    zb_d = nc.dram_tensor("zb_hbm", [128, 2 * D], BF, kind="ExternalInput")
    cb_d = nc.dram_tensor("cb_hbm", [128, CB_TOTAL], BF, kind="ExternalInput")
    cf_d = nc.dram_tensor("cf_hbm", [128, CF_TOTAL], FP, kind="ExternalInput")
    loss_d = nc.dram_tensor("loss", [SPC, 1], FP, kind="ExternalOutput")

    ctx = ExitStack()
    with ctx:
        sem = {n: ctx.enter_context(nc.semaphore(n)) for n in
               ["dzs", "dza", "dcb", "dcf", "dout", "spe", "sv", "sa", "sgp"]}

        def sb(name, shape, dtype=FP):
            return ctx.enter_context(nc.sbuf_tensor(name, shape, dtype))

        zb = sb("zb", [128, 2 * D], BF)    # bf16 z1||z2
        sqb = sb("sqb", [128, 2 * D], BF)  # bf16 z^2
        cb = sb("cb", [128, CB_TOTAL], BF)
        cf = sb("cf", [128, CF_TOTAL])
        eyes = sb("eyes", [128, 64], BF)   # 8x [128,8] bf16 one-hot cols
        statq = sb("statq", [8, 512], BF)  # rows 0-3 sums, 4-7 sumsq
        t1 = sb("t1", [16, 128])
        var16 = sb("var16", [16, 128])
        lnv = sb("lnv", [16, 128])
        arec = sb("arec", [16, 128])
        acz = sb("acz", [16, 256], BF)     # A || C bf16
        tn1 = sb("tn1", [128, 128], BF)
        z1n = sb("z1n", [128, 128], BF)
        tn2 = sb("tn2", [128, 128], BF)
        z2n = sb("z2n", [128, 128], BF)
        w = sb("w", [128, 128], BF)
        u = sb("u", [128, 128], BF)
        junk_v = sb("junk_v", [128, 128], BF)
        junk_s = sb("junk_s", [128, 128], BF)
        colsD = sb("colsD", [128, 8])
        negone = sb("negone", [128, 1])
        a_sb = sb("a_sb", [128, 1])
        junk1c = sb("junk1c", [128, 1])
        fin2 = sb("fin2", [16, 8])
        qs = sb("qs", [16, 8])
        g2c = sb("g2c", [16, 1])
        ec = sb("ec", [16, 1])
        acc1c = sb("acc1c", [16, 1])
        acc2c = sb("acc2c", [16, 1])
        loss16 = sb("loss16", [16, 1])
        junkw = sb("junkw", [1, 4])
        junkw2 = sb("junkw2", [1, 4])

        # PSUM
        warmps = ctx.enter_context(nc.psum_tensor("warmps", [128, 512], FP))
        bankAll = ctx.enter_context(nc.psum_tensor("bankAll", [8, 512], FP))
        statSps = ctx.enter_context(nc.psum_tensor("statSps", [16, 128], FP))
        statQps = ctx.enter_context(nc.psum_tensor("statQps", [16, 128], FP))
        psBC1 = ctx.enter_context(nc.psum_tensor("psBC1", [128, 256], FP))
        psBC2 = ctx.enter_context(nc.psum_tensor("psBC2", [128, 256], FP))
        qfinps = ctx.enter_context(nc.psum_tensor("qfinps", [16, 8], FP))

        z1r = cb[:, CB_Z1R:CB_Z1R + 128]
        z2r = cb[:, CB_Z2R:CB_Z2R + 128]
        amask = cb[:, CB_AMASK:CB_AMASK + 128]
        selz1b = cb[0:16, CB_SEL:CB_SEL + 128]
        selz2b = cb[0:16, CB_SEL2:CB_SEL2 + 128]
        gsel = cf[:, CF_GSEL:CF_GSEL + 16]
        c1 = cf[0:16, CF_C1:CF_C1 + 3]
        c2 = cf[0:16, CF_C2:CF_C2 + 5]
        A_zb = acz[:, 0:128]
        C_zb = acz[:, 128:256]
        statS = statSps[:, :]
        statQ = statQps[:, :]

        blk = [slice(i * 512, (i + 1) * 512) for i in range(4)]

        def eye(m):
            return eyes[:, 8 * m:8 * m + 8]

        def scatS(wn):
            return cb[0:8, CB_SCAT + 16 * wn:CB_SCAT + 16 * wn + 16]

        def scatQ(wn):
            return cb[0:8, CB_SCAT + 64 + 16 * wn:CB_SCAT + 64 + 16 * wn + 16]

        with nc.Block() as block:

            @block.sync
            def _(sync):
                sync.dma_start(zb[:, 0:1024], zb_d[:, 0:1024]).then_inc(sem["dzs"], 16)
                # output DMA once loss16 is drained
                sync.wait_ge(sem["sv"], 31)
                sync.dma_start(loss_d[:], loss16[:]).then_inc(sem["dout"], 16)

            @block.gpsimd
            def _(gp):
                gp.memset(junkw[:], 2.0).then_inc(sem["sgp"])                   # 1
                gp.memset(eyes[:], 0.0).then_inc(sem["sgp"])                    # 2
                for m in range(8):
                    gp.memset(eyes[:, 9 * m:9 * m + 1], 1.0).then_inc(sem["sgp"])  # 3-10
                gp.memset(negone[:], -1.0).then_inc(sem["sgp"])                 # 11
                gp.memset(fin2[:, 3:4], 1.0).then_inc(sem["sgp"])               # 12
                # SWDGE loads for the constant tables (3rd DMA queue)
                gp.dma_start(cb[:], cb_d[:]).then_inc(sem["dcb"], 16)
                gp.dma_start(cf[:], cf_d[:]).then_inc(sem["dcf"], 16)
                gp.wait_ge(sem["dout"], 16)

            @block.scalar
            def _(act):
                act.dma_start(zb[:, 1024:2048], zb_d[:, 1024:2048]).then_inc(sem["dza"], 16)
                # preload the ACT function table during the DMA phase
                act.square(junkw2[:], junkw[:]).then_inc(sem["sa"])             # 1
                act.wait_ge(sem["dzs"], 16)
                act.square(sqb[:, blk[0]], zb[:, blk[0]]).then_inc(sem["sa"])   # 2
                act.square(sqb[:, blk[1]], zb[:, blk[1]]).then_inc(sem["sa"])   # 3
                act.copy(junkw2[:], junkw[:]).then_inc(sem["sa"])               # 4 (spacing)
                # t1 = K1 * S^2 straight out of scatter PSUM
                act.wait_ge(sem["spe"], 12)
                act.activation(t1[:], statS, AF.Square,
                               scale=float(np.sqrt(K1))).then_inc(sem["sa"])    # 5
                act.wait_ge(sem["sv"], 4)
                act.activation(lnv[:], var16[:], AF.Ln).then_inc(sem["sa"])     # 5
                act.copy(junkw2[:], junkw[:]).then_inc(sem["sa"])               # 6 (spacing)
                act.activation(arec[:], lnv[:], AF.Exp,
                               scale=-0.5).then_inc(sem["sa"])                  # 7
                # ---- per-sample squares ----
                act.wait_ge(sem["sv"], 8)
                act.activation(junk_s[:], z1n[:], AF.Square,
                               accum_out=colsD[:, 5:6]).then_inc(sem["sa"])     # 8 Sa
                act.wait_ge(sem["sgp"], 11)
                act.wait_ge(sem["sv"], 12)
                act.activation(u[:], z2n[:], AF.Square, bias=negone[:],
                               scale=a_sb[:]).then_inc(sem["sa"])               # 9 u
                act.activation(junk_s[:], u[:], AF.Square,
                               accum_out=colsD[:, 2:3]).then_inc(sem["sa"])     # 10 Q
                act.copy(junkw2[:], junkw[:]).then_inc(sem["sa"])               # 11 (spacing)
                act.wait_ge(sem["spe"], 19)
                act.copy(qs[:], qfinps[:]).then_inc(sem["sa"])                  # 12
                act.wait_ge(sem["sv"], 19)
                act.activation(g2c[:], fin2[:, 4:5], AF.Square,
                               bias=negone[0:16, :]).then_inc(sem["sa"])        # 13

            @block.vector
            def _(dve):
                dve.wait_ge(sem["dza"], 16)
                dve.scalar_tensor_tensor(
                    sqb[:, blk[2]], zb[:, blk[2]], 1.0, zb[:, blk[2]],
                    op0=AL.bypass, op1=AL.mult).then_inc(sem["sv"])             # 1
                dve.scalar_tensor_tensor(
                    sqb[:, blk[3]], zb[:, blk[3]], 1.0, zb[:, blk[3]],
                    op0=AL.bypass, op1=AL.mult).then_inc(sem["sv"])             # 2
                # stats PSUM -> SBUF (bf16)
                dve.wait_ge(sem["spe"], 8)
                dve.tensor_copy(statq[:], bankAll[:]).then_inc(sem["sv"])       # 3
                # var = K2*Q - t1 (Q straight out of scatter PSUM)
                dve.wait_ge(sem["spe"], 16)
                dve.wait_ge(sem["sa"], 5)
                dve.scalar_tensor_tensor(
                    var16[:], statQ, K2, t1[:],
                    op0=AL.mult, op1=AL.subtract).then_inc(sem["sv"])           # 4
                dve.wait_ge(sem["sa"], 8)
                dve.scalar_tensor_tensor(
                    C_zb, statS, 1.0 / B, arec[:],
                    op0=AL.mult, op1=AL.mult).then_inc(sem["sv"])               # 5
                dve.tensor_copy(A_zb, arec[:]).then_inc(sem["sv"])              # 6
                # ---- normalize + per-sample products ----
                dve.wait_ge(sem["spe"], 17)
                dve.wait_ge(sem["dcb"], 16)
                dve.tensor_tensor(tn1[:], z1r, psBC1[:, 0:128], AL.mult).then_inc(sem["sv"])  # 7
                dve.tensor_tensor(z1n[:], tn1[:], psBC1[:, 128:256],
                                  AL.subtract).then_inc(sem["sv"])              # 7
                dve.scalar_tensor_tensor(
                    junk_v[:], z1n[:], 1.0, amask, op0=AL.bypass, op1=AL.mult,
                    accum_out=colsD[:, 3:4]).then_inc(sem["sv"])                # 8
                dve.wait_ge(sem["spe"], 18)
                dve.tensor_tensor(tn2[:], z2r, psBC2[:, 0:128], AL.mult).then_inc(sem["sv"])  # 9
                dve.tensor_tensor(z2n[:], tn2[:], psBC2[:, 128:256],
                                  AL.subtract).then_inc(sem["sv"])              # 10
                dve.stream_shuffle(a_sb[:], colsD[:, 3:4],
                                   [8 * (i // 8) for i in range(32)]).then_inc(sem["sv"])  # 11
                dve.scalar_tensor_tensor(
                    w[:], z1n[:], 1.0, z2n[:], op0=AL.bypass, op1=AL.mult,
                    accum_out=colsD[:, 0:1]).then_inc(sem["sv"])                # 12 R
                dve.scalar_tensor_tensor(
                    junk_v[:], z2n[:], 1.0, amask, op0=AL.bypass, op1=AL.mult,
                    accum_out=colsD[:, 4:5]).then_inc(sem["sv"])                # 13 v
                dve.scalar_tensor_tensor(
                    junk_v[:], z2n[:], 1.0, z2n[:], op0=AL.bypass, op1=AL.mult,
                    accum_out=colsD[:, 6:7]).then_inc(sem["sv"])                # 14 Sv
                dve.scalar_tensor_tensor(
                    junk_v[:], w[:], 1.0, w[:], op0=AL.bypass, op1=AL.mult,
                    accum_out=colsD[:, 1:2]).then_inc(sem["sv"])                # 15 P
                dve.tensor_tensor(colsD[:, 7:8], colsD[:, 3:4], colsD[:, 3:4],
                                  AL.mult).then_inc(sem["sv"])                  # 16 a^2
                dve.drain().then_inc(sem["sv"])                                 # 17
                # ---- finals ----
                dve.wait_ge(sem["spe"], 19)
                dve.wait_ge(sem["sa"], 13)
                dve.wait_ge(sem["sgp"], 12)
                dve.tensor_tensor(fin2[:, 4:5], qs[:, 3:4], qs[:, 4:5],
                                  AL.mult).then_inc(sem["sv"])                  # 18 d
                dve.tensor_tensor(ec[:], qs[:, 7:8], qs[:, 5:6],
                                  AL.subtract).then_inc(sem["sv"])              # 19 a2-Sa
                dve.scalar_tensor_tensor(
                    fin2[:, 5:8], qs[:, 0:3], 1.0, c1, op0=AL.bypass,
                    op1=AL.mult, accum_out=acc1c[:]).then_inc(sem["sv"])        # 20 acc1
                dve.tensor_tensor(fin2[:, 1:2], fin2[:, 4:5], fin2[:, 4:5],
                                  AL.mult).then_inc(sem["sv"])                  # 21 d^2
                dve.tensor_tensor(fin2[:, 2:3], ec[:], qs[:, 6:7],
                                  AL.mult).then_inc(sem["sv"])                  # 22 f
                dve.wait_ge(sem["sa"], 14)
                dve.tensor_tensor(fin2[:, 0:1], g2c[:], g2c[:],
                                  AL.mult).then_inc(sem["sv"])                  # 23 g2^2
                dve.tensor_tensor(junk1c[:], negone[:], negone[:],
                                  AL.mult).then_inc(sem["sv"])                  # 24 (spacing)
                dve.tensor_tensor(junk1c[:], negone[:], negone[:],
                                  AL.mult).then_inc(sem["sv"])                  # 25 (spacing)
                dve.scalar_tensor_tensor(
                    qs[:, 0:5], fin2[:, 0:5], 1.0, c2, op0=AL.bypass,
                    op1=AL.mult, accum_out=acc2c[:]).then_inc(sem["sv"])        # 26 acc2
                dve.tensor_tensor(junk1c[:], negone[:], negone[:],
                                  AL.mult).then_inc(sem["sv"])                  # 27 (spacing)
                dve.tensor_tensor(junk1c[:], negone[:], negone[:],
                                  AL.mult).then_inc(sem["sv"])                  # 28 (spacing)
                dve.tensor_tensor(loss16[:], acc1c[:], acc2c[:],
                                  AL.add).then_inc(sem["sv"])                   # 29
                dve.drain().then_inc(sem["sv"])                                 # 30

            @block.tensor
            def _(pe):
                # p-state warmups on garbage SBUF -> scratch PSUM: 3 wide to
                # fill the pipe, then narrow trickles to hold the clock high
                # without delaying the first real matmul.
                for i in range(3):
                    pe.matmul(warmps[0:8, :], cb[:, 0:8], cb[:, 128:640],
                              start=True, stop=True,
                              skip_group_check=True)
                for i in range(16):
                    pe.matmul(warmps[0:8, 0:128], cb[:, 0:8], cb[:, 128:256],
                              start=True, stop=True,
                              skip_group_check=True)
                # column sums (rows 0-3) + sumsq (rows 4-7), one bank
                pe.wait_ge(sem["sgp"], 10)
                pe.wait_ge(sem["dzs"], 16)
                pe.matmul(bankAll[:], eye(0), zb[:, blk[0]], start=True, stop=False,
                          skip_group_check=True).then_inc(sem["spe"])           # 1
                pe.matmul(bankAll[:], eye(1), zb[:, blk[1]], start=False, stop=False,
                          skip_group_check=True).then_inc(sem["spe"])           # 2
                pe.wait_ge(sem["dza"], 16)
                pe.matmul(bankAll[:], eye(2), zb[:, blk[2]], start=False, stop=False,
                          skip_group_check=True).then_inc(sem["spe"])           # 3
                pe.matmul(bankAll[:], eye(3), zb[:, blk[3]], start=False, stop=False,
                          skip_group_check=True).then_inc(sem["spe"])           # 4
                pe.wait_ge(sem["sa"], 2)
                pe.matmul(bankAll[:], eye(4), sqb[:, blk[0]], start=False, stop=False,
                          skip_group_check=True).then_inc(sem["spe"])           # 5
                pe.wait_ge(sem["sa"], 3)
                pe.matmul(bankAll[:], eye(5), sqb[:, blk[1]], start=False, stop=False,
                          skip_group_check=True).then_inc(sem["spe"])           # 6
                pe.wait_ge(sem["sv"], 1)
                pe.matmul(bankAll[:], eye(6), sqb[:, blk[2]], start=False, stop=False,
                          skip_group_check=True).then_inc(sem["spe"])           # 7
                pe.wait_ge(sem["sv"], 2)
                pe.matmul(bankAll[:], eye(7), sqb[:, blk[3]], start=False, stop=True,
                          skip_group_check=True).then_inc(sem["spe"])           # 8
                # scatter [8,512] -> 2x [16,128]
                pe.wait_ge(sem["dcb"], 16)
                pe.wait_ge(sem["sv"], 3)
                pe.wait_ge(sem["sa"], 4)
                for wn in range(4):
                    pe.matmul(statSps[:], scatS(wn),
                              statq[:, 128 * wn:128 * wn + 128],
                              start=(wn == 0), stop=(wn == 3),
                              skip_group_check=True).then_inc(sem["spe"])       # 9-12
                for wn in range(4):
                    pe.matmul(statQps[:], scatQ(wn),
                              statq[:, 128 * wn:128 * wn + 128],
                              start=(wn == 0), stop=(wn == 3),
                              skip_group_check=True).then_inc(sem["spe"])       # 13-16
                # broadcast A||C to the (sample, chunk) layout
                pe.wait_ge(sem["sv"], 6)
                pe.matmul(psBC1[:], selz1b, acz[:], start=True,
                          stop=True).then_inc(sem["spe"])                       # 17
                pe.matmul(psBC2[:], selz2b, acz[:], start=True, stop=True,
                          skip_group_check=True).then_inc(sem["spe"])           # 18
                # group-reduce: collapse 8 chunk-rows per sample
                pe.wait_ge(sem["dcf"], 16)
                pe.wait_ge(sem["sv"], 18)
                pe.wait_ge(sem["sa"], 12)
                pe.matmul(qfinps[:], gsel, colsD[:], start=True,
                          stop=True).then_inc(sem["spe"])                       # 19

    return nc


def _host_inputs(z1, z2):
    """Per-core input maps (sharding glue)."""
    import ml_dtypes

    z1 = np.ascontiguousarray(z1, np.float32)
    z2 = np.ascontiguousarray(z2, np.float32)
    zb_full = np.concatenate([z1, z2], axis=1).astype(ml_dtypes.bfloat16)

    cb_base = np.zeros((128, CB_TOTAL), np.float32)
    for m in range(128):
        cb_base[m % 8, CB_SEL + m] = 1.0        # selz1b (reads A/C rows 0-7)
        cb_base[8 + m % 8, CB_SEL2 + m] = 1.0   # selz2b (reads A/C rows 8-15)
    for wn in range(4):
        for g in range(4):
            # scatter lhsT_w: S block g / Q block g -> chunk row 4g+w
            cb_base[g, CB_SCAT + 16 * wn + 4 * g + wn] = 1.0
            cb_base[4 + g, CB_SCAT + 64 + 16 * wn + 4 * g + wn] = 1.0

    cf_base = np.zeros((128, CF_TOTAL), np.float32)
    for m in range(128):
        cf_base[m, CF_GSEL + m // 8] = 1.0      # gsel
    cf_base[0:16, CF_C1:CF_C1 + 3] = np.array(
        [-2.0, 1.0 - LAM, LAM], np.float32)
    cf_base[0:16, CF_C2:CF_C2 + 5] = np.array(
        [1.0 - LAM, LAM - 3.0, -LAM, float(D - 2), 6.0], np.float32)

    in_maps = []
    for c in range(NCORES):
        rows = slice(c * SPC, (c + 1) * SPC)
        cbc = cb_base.copy()
        cbc[:, CB_Z1R:CB_Z1R + 128] = z1[rows].reshape(128, 128)
        cbc[:, CB_Z2R:CB_Z2R + 128] = z2[rows].reshape(128, 128)
        for s in range(SPC):
            cbc[s * 8, CB_AMASK + c * SPC + s] = 1.0
        in_maps.append({
            "zb_hbm": zb_full,
            "cb_hbm": np.ascontiguousarray(cbc.astype(ml_dtypes.bfloat16)),
            "cf_hbm": np.ascontiguousarray(cf_base),
        })
    return in_maps


_cached_nc = None


def run(z1, z2, trace=False, **kwargs):
    global _cached_nc
    if _cached_nc is None:
        _cached_nc = build_program()
    in_maps = _host_inputs(z1, z2)
    res = run_bass_kernel_spmd(
        _cached_nc, in_maps, core_ids=list(range(NCORES)), trace=trace, **kwargs)
    out = np.concatenate([res.results[c]["loss"][:, 0] for c in range(NCORES)])
    return out.astype(np.float32), res


def kernel(z1, z2):
    out, _ = run(z1, z2, trace=False)
    return out
